# revision 35
# baseline (speedup 1.0000x reference)
"""Trainium2 Bass kernel for a cross-attention block with 3D-coordinate RoPE.

Module: q/k/v projections of x [B,Tq,D] against memory [B,Tk,D], 3D-coord
rotary embedding on q/k, softmax(q k^T / sqrt(Hd)) v, output projection.
B=2, Tq=1024, Tk=2048, D=1536, 16 heads x 96.

Sharding: 8 cores = (2 batches) x (4 head-groups of 4 heads). Each core
computes its heads end-to-end plus a partial output projection; the host
sums the 4 partials per batch. Biases bv/bo are folded in on the host
(attention rows sum to one), bq/bk are added on-device during PSUM
eviction.

Layout: feature-major ("transposed") on device. Scores are computed
transposed (S^T = k q^T) so the PV matmul needs no on-chip transposes;
softmax denominators come from a ones-column appended to v; the
per-query normalization is broadcast across partitions with a K=1
ones-vector matmul + full-lane approximate reciprocal.

Schedule notes (v4, ~210us vs 252us for v3):
 - Phase order is Q -> K -> fused(V+scores+PV+O).  K right after Q means
   the ACT engine's ~93us of exp work (64 chunks x ~1.45us, the
   co-critical resource) starts at ~35us instead of ~110us and hides
   almost entirely under PE matmul phases.
 - Bulk inputs stream on sync in consumption order as >=3KB/partition
   batched transfers (2KB transfers measured ~260GB/s vs ~390GB/s for
   4KB+): wq halves, x triples, trig tables, wk, mem pairs, wv.  wo
   loads after the K phase so its SBUF never competes with the front.
 - PE warm-up matmuls + ACT table preload run during the initial DMA
   flight (HAM clock gate reaches 8/8; first real activation would
   otherwise pay the ~1.3us table load).  Free-running dummy matmuls at
   the known DMA-wait points (x-transfer boundaries, the Q->K handoff,
   memory-arrival waits in K half 0) keep the clock gate warm through
   the front, removing ~5us of 1.2GHz cold-matmul penalty.
 - Q and K projections are PACKED (three 128-row matmuls instead of four
   96-row ones); per-head tiles are recovered with SBUF->SBUF shift DMAs
   (engines are lane-locked, DMA crosses partitions).
 - K runs chunk-outer over the arriving memory stream (half 0), then
   PE-bound (half 1) with the first 8 head-0 score chunks interleaved in
   the 2 spare PSUM banks.  Unpack/rope chains are emitted AFTER each
   half's PSUM evictions (engines are strict FIFO - rope ops queued
   ahead of evictions would stall the next phase's bank reuse), head 0
   first so the early scores can issue.
 - The fused tail interleaves all 16 V row-blocks (front-loaded so every
   head's PV pass can finish early), 56 score chunks, 64 PV
   accumulations and the softmax-normalization drains in one static
   schedule with an exp-aware pacing gate; the output projection is
   software-pipelined (i0/i1 accumulations for four m-tiles emitted
   before any i2) so the last head's DVE drain hides under matmuls.
 - RoPE uses an SBUF->SBUF DMA partition swap instead of a PE
   permutation matmul; the multiply/add chain runs on DVE in fp16.
 - The per-query normalization multiplies run all-fp16 (mixed
   fp16*fp32 on DVE measured 1459ns vs ~550ns all-fp16), split
   DVE/Pool.
 - Trig tables are fp16; output is fp16 (host accumulates in fp32).
"""

import os
import sys

sys.path.insert(0, "/opt/trn_rl_repo")

import numpy as np
import ml_dtypes
from contextlib import ExitStack

import concourse.bass as bass
import concourse.tile as tile
from concourse import bacc, mybir
from concourse.bass_utils import run_bass_kernel_spmd

# ---------------------------------------------------------------- constants
B = 2
TQ = 1024
TK = 2048
D = 1536
NH = 16
HD = 96
ROPE_HALF = HD // 2           # 48
FREQ_PER_AXIS = ROPE_HALF // 3  # 16
ROPE_BASE = 10000.0
NH_CORE = 4                   # heads per core
HG = NH_CORE * HD             # 384 features per core
KC = D // 128                 # 12 contraction chunks
MTILES = D // 128             # 12 output-row tiles of the o-projection
SCALE = 1.0 / float(np.sqrt(HD))
N_CORES = 8
VW = HD + 1                   # 97: head-dim + ones column

F32 = mybir.dt.float32

_MM_DT_NAME = os.environ.get("KMM_DTYPE", "f16")
_DT = {"bf16": mybir.dt.bfloat16, "f16": mybir.dt.float16}
_NP = {"bf16": ml_dtypes.bfloat16, "f16": np.float16}
QK_DT = PV_DT = _DT[_MM_DT_NAME]
QK_NP = PV_NP = _NP[_MM_DT_NAME]


# ---------------------------------------------------------------- bass build
def _build_nc():
    nc = bacc.Bacc(trn_type="TRN2", target_bir_lowering=False, debug=False)

    io = {}
    def dram_in(name, shape, dt):
        io[name] = nc.dram_tensor(name, list(shape), dt, kind="ExternalInput").ap()
    dram_in("xT", [D, TQ], QK_DT)
    dram_in("memT", [D, TK], QK_DT)
    dram_in("wqT", [D, HG], QK_DT)  # columns of Wq^T for this head group
    dram_in("wkT", [D, HG], QK_DT)
    dram_in("wvT", [D, HG], PV_DT)
    dram_in("woT", [HG, D], PV_DT)  # rows of Wo^T for this head group
    dram_in("bqP", [128, 3], F32)   # biases packed by 128-row psum tile
    dram_in("bkP", [128, 3], F32)
    dram_in("cqE", [HD, TQ], QK_DT)  # cos table, feature-major, q side
    dram_in("sqE", [HD, TQ], QK_DT)  # sign-folded sin table, q side
    dram_in("ckE", [HD, TK], QK_DT)
    dram_in("skE", [HD, TK], QK_DT)
    oT = nc.dram_tensor("oT", [D, TQ], PV_DT, kind="ExternalOutput").ap()

    with tile.TileContext(nc) as tc, ExitStack() as ctx:
        _body(ctx, tc, io, oT)
    nc.compile()
    return nc


def _body(ctx, tc, io, oT):
    nc = tc.nc
    P = 128
    NKC = TK // P
    Exp = mybir.ActivationFunctionType.Exp
    Ident = mybir.ActivationFunctionType.Identity

    const = ctx.enter_context(tc.tile_pool(name="const", bufs=1))
    resident = ctx.enter_context(tc.tile_pool(name="resident", bufs=1))

    ones1_t = const.tile([1, P], PV_DT, name="ones1_t")
    bq_t = const.tile([P, 3], F32, name="bq_t")
    bk_t = const.tile([P, 3], F32, name="bk_t")
    cq_t = const.tile([HD, TQ], QK_DT, name="cq_t")
    sq_t = const.tile([HD, TQ], QK_DT, name="sq_t")
    ck_t = const.tile([HD, TK], QK_DT, name="ck_t")
    sk_t = const.tile([HD, TK], QK_DT, name="sk_t")
    wv_all = const.tile([P, KC, HG], PV_DT, name="wv_all")
    qT = [resident.tile([HD, TQ], QK_DT, name=f"qT{h}", tag=f"qT{h}")
          for h in range(NH_CORE)]
    kT = [resident.tile([HD, TK], QK_DT, name=f"kT{h}", tag=f"kT{h}")
          for h in range(NH_CORE)]
    vst = [resident.tile([P, NH_CORE * VW], PV_DT, name=f"vst{m}", tag=f"vst{m}")
           for m in range(NKC)]
    tmp_pool = ctx.enter_context(tc.tile_pool(name="tmp_pool", bufs=4))
    # exp backlog: scores run far ahead of PV consumption so the ACT
    # engine's 64 x 1.45us of exp work hides under PE matmul phases
    p_pool = ctx.enter_context(tc.tile_pool(name="p_pool", bufs=17))
    # full memory^T resident: loaded once as ONE tile so DMA arrives in
    # 8KB-per-partition batched transfers (2KB transfers measured ~260GB/s,
    # 4KB+ ~390GB/s)
    mem_pool = ctx.enter_context(tc.tile_pool(name="mem_pool", bufs=1))
    mem_all = mem_pool.tile([P, KC, TK], QK_DT, name="mem_all")
    memR = [mem_all[:, c, :] for c in range(KC)]
    # rotating swap-source tiles for RoPE + packed-projection staging,
    # freed after the rope phase (LIFO: wk closes first, then sw, then mem)
    sw_stack = ExitStack()
    sw_pool = sw_stack.enter_context(tc.tile_pool(name="sw_pool", bufs=2))
    stage_pool = sw_stack.enter_context(tc.tile_pool(name="stage", bufs=3))
    # wk lives in its own pool freed after the K projection
    wk_stack = ExitStack()
    wk_pool = wk_stack.enter_context(tc.tile_pool(name="wk_pool", bufs=1))
    wk_all = wk_pool.tile([P, KC, HG], QK_DT, name="wk_all")

    # ---- PE warm-up: matmuls on memset tiles while input DMAs fly --------
    wz = const.tile([P, 512], QK_DT, name="wz")
    with ExitStack() as wctx:
        warm_ps = wctx.enter_context(
            tc.tile_pool(name="warm_ps", bufs=1, space="PSUM"))
        nc.gpsimd.memset(wz[:], 0.0)
        wps = warm_ps.tile([P, 512], F32, name="wps", tag="wps")
        nc.scalar.activation(wz[0:1, 0:1], wz[0:1, 0:1], Exp)
        nc.scalar.activation(wz[0:1, 1:2], wz[0:1, 1:2], Ident)
        for _ in range(12):
            nc.tensor.matmul(wps[:], wz[:, 0:P], wz[:], start=True, stop=True)

    # ones for the denominator broadcast + v ones-columns (no DMA needed)
    nc.gpsimd.memset(ones1_t[:], 1.0)
    for m in range(NKC):
        ones_cols = vst[m].rearrange("p (h c) -> p h c", c=VW)[:, :, HD:HD + 1]
        nc.gpsimd.memset(ones_cols, 1.0)

    # ---- input DMA issue ----------------------------------------------
    # Bulk stream on sync in consumption order, batched into >=3KB/part
    # transfers.  Trig tables ride the sync stream AFTER x so the x pairs
    # (which pace the Q projection) get full HBM bandwidth; only the tiny
    # biases go on the scalar queue.
    nc.scalar.dma_start(out=bq_t[:], in_=io["bqP"][:])
    nc.scalar.dma_start(out=bk_t[:], in_=io["bkP"][:])

    def rope(dst, cE, sE, lo, width, dma_eng=None):
        """RoPE on dst[:, lo:lo+width] via DMA partition swap + DVE fp16.

        swp[j] = dst[(j+48) % 96]; dst <- dst*cE + swp*sE (sE sign-folded).
        """
        if dma_eng is None:
            dma_eng = nc.gpsimd
        sl = slice(lo, lo + width)
        swp = sw_pool.tile([HD, width], QK_DT, name="swp", tag="sw")
        dma_eng.dma_start(out=swp[0:ROPE_HALF, :],
                          in_=dst[ROPE_HALF:HD, sl])
        dma_eng.dma_start(out=swp[ROPE_HALF:HD, :],
                          in_=dst[0:ROPE_HALF, sl])
        t1 = tmp_pool.tile([HD, width], QK_DT, name="t1", tag="tmp")
        t2 = tmp_pool.tile([HD, width], QK_DT, name="t2", tag="tmp")
        nc.vector.tensor_mul(t1[:], dst[:, sl], cE[:, sl])
        nc.vector.tensor_mul(t2[:], swp[:], sE[:, sl])
        nc.vector.tensor_add(dst[:, sl], t1[:], t2[:])

    # packed-projection unpack: psum rows are packed features 128*m+p;
    # head h rows live at packed rows 96h..96h+95.  DMA shifts them to
    # partition 0 of the per-head tiles (engines are lane-locked; DMA
    # crosses partitions freely).
    SEGS = [(0, 0, 0, 0, 96),
            (1, 0, 0, 96, 32), (1, 32, 1, 0, 64),
            (2, 0, 1, 64, 64), (2, 64, 2, 0, 32),
            (3, 0, 2, 32, 96)]

    def unpack_shift(dstT, stg, col_dst, width):
        for (h, r0, m3, s0, nr) in SEGS:
            nc.gpsimd.dma_start(
                out=dstT[h][r0:r0 + nr, col_dst:col_dst + width],
                in_=stg[m3][s0:s0 + nr, 0:width])

    PTS = {}

    def emit_schunk(h, kc, pool):
        st = pool.tile([P, TQ], F32, name="st", tag="s")
        lhs = kT[h][:, kc * P:(kc + 1) * P]
        nc.tensor.matmul(st[:, 0:512], lhs, qT[h][:, 0:512])
        nc.tensor.matmul(st[:, 512:1024], lhs, qT[h][:, 512:1024])
        pt = p_pool.tile([P, TQ], PV_DT, name="pt", tag="pt")
        nc.scalar.activation(pt[:], st[:], Exp, scale=SCALE)
        PTS[(h, kc)] = pt

    def emit_vblock(mg, pool):
        """V row-block mg: 12 accumulation matmuls + DVE eviction into vst
        (ACT is kept clear for the exp pipeline)."""
        psv = pool.tile([P, HG], F32, name=f"psv{mg}", tag="psv")
        for c in range(KC):
            nc.tensor.matmul(
                psv[:], memR[c][:, mg * P:(mg + 1) * P], wv_all[:, c, :],
                start=(c == 0), stop=(c == KC - 1))
        dst = vst[mg].rearrange("p (h c) -> p h c", c=VW)[:, :, 0:HD]
        src = psv.rearrange("p (h c) -> p h c", c=HD)
        nc.vector.tensor_copy(dst, src)

    # sK pool opens early: its 2 banks host K1's early score chunks later,
    # and during the DMA-paced front they host free-running dummy matmuls
    # that keep the HAM clock gate at 8/8 across the x/mem stall points
    # (re-throttled matmuls run at 1.2GHz; ~35 cold matmuls measured).
    sK_stack = ExitStack()
    sK_pool = sK_stack.enter_context(
        tc.tile_pool(name="sK", bufs=1, space="PSUM"))

    def dummy_mms(n):
        dmy = sK_pool.tile([P, TQ], F32, name="dmy", tag="s")
        for _ in range(n):
            nc.tensor.matmul(dmy[:, 0:512], wz[:, 0:P], wz[:],
                             start=True, stop=True, skip_group_check=True)

    # ---- phase Q: packed q^T projection (6 banks), chunk-outer over x ----
    with ExitStack() as qctx:
        psq_pool = qctx.enter_context(
            tc.tile_pool(name="psq", bufs=3, space="PSUM"))
        xq_pool = qctx.enter_context(tc.tile_pool(name="xq", bufs=2))
        wq_pool = qctx.enter_context(tc.tile_pool(name="wq", bufs=1))
        wq_all = wq_pool.tile([P, KC, HG], QK_DT, name="wq_all")
        psq = [psq_pool.tile([P, TQ], F32, name=f"psq{m3}", tag="psq")
               for m3 in range(3)]
        # bulk stream: wq, x (pairs), trig tables, wk, mem (pairs), wv, wo
        # — all transfers >=2KB/partition, in consumption order
        for half_w in range(2):
            nc.sync.dma_start(
                out=wq_all[:, 6 * half_w:6 * (half_w + 1), :],
                in_=io["wqT"][768 * half_w:768 * (half_w + 1), :]
                .rearrange("(c p) h -> p c h", p=P))
        xps = []
        for j in range(KC // 3):
            xp = xq_pool.tile([P, 3, TQ], QK_DT, name=f"xp{j}", tag="xp")
            nc.sync.dma_start(
                out=xp[:],
                in_=io["xT"][384 * j:384 * (j + 1), :]
                .rearrange("(c p) t -> p c t", p=P))
            xps.append(xp)
        nc.sync.dma_start(out=cq_t[:], in_=io["cqE"][:])
        nc.sync.dma_start(out=sq_t[:], in_=io["sqE"][:])
        nc.sync.dma_start(
            out=wk_all[:],
            in_=io["wkT"][:].rearrange("(c p) h -> p c h", p=P))
        nc.sync.dma_start(out=ck_t[:], in_=io["ckE"][:])
        nc.sync.dma_start(out=sk_t[:], in_=io["skE"][:])
        for j in range(KC // 2):
            nc.sync.dma_start(
                out=mem_all[:, 2 * j:2 * j + 2, :],
                in_=io["memT"][256 * j:256 * (j + 1), :]
                .rearrange("(c p) t -> p c t", p=P))
        nc.sync.dma_start(
            out=wv_all[:],
            in_=io["wvT"][:].rearrange("(c p) h -> p c h", p=P))
        for c in range(KC):
            if c in (3, 6, 9):
                dummy_mms(2)
            for m3 in range(3):
                lhs = wq_all[:, c, m3 * P:(m3 + 1) * P]
                for n in range(2):
                    nc.tensor.matmul(
                        psq[m3][:, n * 512:(n + 1) * 512],
                        lhs, xps[c // 3][:, c % 3, n * 512:(n + 1) * 512],
                        start=(c == 0), stop=(c == KC - 1))
        dummy_mms(6)
        qP = [stage_pool.tile([P, TQ], QK_DT, name=f"qP{m3}", tag="stg")
              for m3 in range(3)]
        for m3 in range(3):
            if m3 % 2 == 0:
                nc.vector.tensor_scalar_add(qP[m3][:], psq[m3][:],
                                            bq_t[:, m3:m3 + 1])
            else:
                nc.scalar.activation(qP[m3][:], psq[m3][:], Ident,
                                     bias=bq_t[:, m3:m3 + 1])
        unpack_shift(qT, qP, 0, TQ)

    # ---- phase K: packed k^T, two column halves ------------------------
    # Half 0 runs chunk-outer over the arriving memory stream; half 1 is
    # PE-bound and interleaves the first head-0 score chunks (+exp) so the
    # ACT engine starts its 64-chunk exp workload ~75us earlier than a
    # V-before-K ordering would allow.  The rope/unpack work is emitted
    # AFTER each half's PSUM evictions (DVE is strict FIFO — rope TT ops
    # queued ahead of the evictions would stall the next half's matmuls),
    # pipelined per head so head 0 ropes first for the early scores.
    NPRE = 8

    def unpack_head(dstT, stg, col_dst, width, h):
        for (hs, r0, m3, s0, nr) in SEGS:
            if hs == h:
                nc.gpsimd.dma_start(
                    out=dstT[h][r0:r0 + nr, col_dst:col_dst + width],
                    in_=stg[m3][s0:s0 + nr, 0:width])

    with ExitStack() as kctx:
        psk_pool = kctx.enter_context(
            tc.tile_pool(name="psk", bufs=6, space="PSUM"))
        for half in range(2):
            base = half * 1024
            psk = [psk_pool.tile([P, 512], F32, name=f"psk{half}_{i}",
                                 tag="psk") for i in range(6)]
            # first touch of each bank gates on the previous phase's
            # eviction; order c==0 so the ACT-evicted banks (psk[2], [3])
            # come last and hide behind the DVE-evicted ones
            ORD0 = [(0, 0), (1, 0), (1, 1), (2, 1), (0, 1), (2, 0)]
            for c in range(KC):
                pairs = (ORD0 if (c == 0 and half == 0)
                         else [(m3, qq) for m3 in range(3) for qq in range(2)])
                for (m3, qq) in pairs:
                    lhs = wk_all[:, c, m3 * P:(m3 + 1) * P]
                    col = base + qq * 512
                    nc.tensor.matmul(
                        psk[qq * 3 + m3][:], lhs,
                        memR[c][:, col:col + 512],
                        start=(c == 0), stop=(c == KC - 1))
                if half == 0 and c in (3, 5, 7, 9):
                    dummy_mms(4 if c in (5, 7) else 2)
                if half == 1 and 4 <= c <= 3 + NPRE:
                    emit_schunk(0, c - 4, sK_pool)
            kP = [stage_pool.tile([P, TQ], QK_DT, name=f"kP{half}_{m3}",
                                  tag="stg") for m3 in range(3)]
            for qq in range(2):
                for m3 in range(3):
                    i = qq * 3 + m3
                    dst = kP[m3][:, qq * 512:(qq + 1) * 512]
                    if i % 2 == 0:
                        nc.vector.tensor_scalar_add(dst, psk[i][:],
                                                    bk_t[:, m3:m3 + 1])
                    else:
                        nc.scalar.activation(dst, psk[i][:], Ident,
                                             bias=bk_t[:, m3:m3 + 1])
            if half == 0:
                # interleave q and k ropes head-by-head: head 0 (q then k)
                # completes just before the half-1 loop reaches its first
                # score chunk at c=4
                rope(qT[0], cq_t, sq_t, 0, TQ, dma_eng=nc.gpsimd)
                unpack_head(kT, kP, 0, 1024, 0)
                rope(kT[0], ck_t, sk_t, 0, 1024, dma_eng=nc.gpsimd)
                for h in range(1, NH_CORE):
                    rope(qT[h], cq_t, sq_t, 0, TQ, dma_eng=nc.sync)
                for h in range(1, NH_CORE):
                    unpack_head(kT, kP, 0, 1024, h)
                    rope(kT[h], ck_t, sk_t, 0, 1024, dma_eng=nc.gpsimd)
            else:
                for h in range(NH_CORE):
                    unpack_head(kT, kP, 1024, 1024, h)
                    rope(kT[h], ck_t, sk_t, 1024, 1024, dma_eng=nc.sync)
    wk_stack.close()
    sw_stack.close()
    sK_stack.close()

    # wo loads here (sync queue is free) so its SBUF never competes with
    # the front-phase pools
    wo_pool = ctx.enter_context(tc.tile_pool(name="wo_pool", bufs=1))
    wo_t = [wo_pool.tile([P, D], PV_DT, name=f"wo_t{i}", tag=f"wo_t{i}")
            for i in range(3)]
    for i in range(3):
        nc.sync.dma_start(out=wo_t[i][:], in_=io["woT"][i * P:(i + 1) * P, :])

    # ---- fused attention: V blocks + scores/exp + PV in one pipeline -----
    # All 16 V row-blocks, the remaining 56 score chunks, and all 64 PV
    # accumulations interleave so no engine drains between phases.
    s_stack = ExitStack()
    s_ps = s_stack.enter_context(tc.tile_pool(name="s_ps", bufs=2,
                                              space="PSUM"))
    flat_stack = ExitStack()
    psv_pool = flat_stack.enter_context(
        tc.tile_pool(name="psv", bufs=2, space="PSUM"))
    pv_ps = flat_stack.enter_context(
        tc.tile_pool(name="pv_ps", bufs=2, space="PSUM"))
    aout_pool = ctx.enter_context(tc.tile_pool(name="aout_pool", bufs=2))
    aN_pool = ctx.enter_context(tc.tile_pool(name="aN_pool", bufs=1))
    ot_pool = ctx.enter_context(tc.tile_pool(name="ot_pool", bufs=2))

    # aoutN stacked as 3 tiles of 128 partitions (heads packed) so the
    # o-projection contracts in 3 chunks of 128 instead of 4 of 96
    aN = [aN_pool.tile([P, TQ], PV_DT, name=f"aN{i}", tag=f"aN{i}")
          for i in range(3)]
    # per-head write segments: (tile, tile_row0, head_row0, nrows)
    _SEG = {0: [(0, 0, 0, 96)],
            1: [(0, 96, 0, 32), (1, 0, 32, 32), (1, 32, 64, 32)],
            2: [(1, 64, 0, 64), (2, 0, 64, 32)],
            3: [(2, 32, 0, 32), (2, 64, 32, 32), (2, 96, 64, 32)]}

    pvs = {}

    def finish_head(h):
        pv0, pv1 = pvs.pop(h)
        aout = aout_pool.tile([VW, TQ], PV_DT, name="aout", tag="aout")
        # denominator row straight from PSUM so the broadcast matmul does
        # not serialize behind the aout eviction; aout halves split across
        # DVE and ACT so the head drain is ~2x shorter
        den1 = tmp_pool.tile([1, TQ], PV_DT, name="den1", tag="den1",
                     bufs=2)
        nc.vector.tensor_copy(den1[:, 0:512], pv0[HD:HD + 1, :])
        nc.vector.tensor_copy(den1[:, 512:1024], pv1[HD:HD + 1, :])
        nc.vector.tensor_copy(aout[:, 0:512], pv0[:])
        nc.scalar.copy(aout[:, 512:1024], pv1[:])
        denB = s_ps.tile([P, TQ], F32, name="denB", tag="s")
        for n in range(2):
            nc.tensor.matmul(denB[:, n * 512:(n + 1) * 512], ones1_t[:],
                             den1[:, n * 512:(n + 1) * 512])
        recB = tmp_pool.tile([HD, TQ], F32, name="recB", tag="tmp")
        nc.vector.reciprocal_approx_fast(out=recB[:], in_=denB[0:HD, :])
        # fp16 reciprocal: the aN multiplies then run in the DVE 2x path
        # (mixed fp16*fp32 operands measured 1459ns vs ~550ns all-fp16),
        # with one segment offloaded to the Pool engine
        recH = tmp_pool.tile([HD, TQ], PV_DT, name="recH", tag="recH",
                             bufs=2)
        nc.vector.tensor_copy(recH[:], recB[:])
        pool_done = False
        for (ti, tr, hr, nr) in _SEG[h]:
            eng = nc.vector
            if ti == 2 and not pool_done:
                eng = nc.gpsimd
                pool_done = True
            eng.tensor_mul(aN[ti][tr:tr + nr, :],
                           aout[hr:hr + nr, :], recH[hr:hr + nr, :])

    def emit_pv(h, kc):
        if kc == 0:
            pvs[h] = (
                pv_ps.tile([VW, 512], F32, name=f"pv{h}0", tag="pv"),
                pv_ps.tile([VW, 512], F32, name=f"pv{h}1", tag="pv"))
        pv0, pv1 = pvs[h]
        pt = PTS.pop((h, kc))
        vl = vst[kc][:, h * VW:(h + 1) * VW]
        first, last = (kc == 0), (kc == NKC - 1)
        nc.tensor.matmul(pv0[:], vl, pt[:, 0:512], start=first, stop=last)
        nc.tensor.matmul(pv1[:], vl, pt[:, 512:1024], start=first, stop=last)
        if last:
            finish_head(h)

    # score-chunk emission order: half-0 columns (roped first) for heads
    # 1-3 lead, then half-1 columns; head 0's half-0 chunks were prescored.
    # PV consumption is strictly head-major, so its exp deps follow
    # exp_pos (the chunk's index in overall exp-emission order).
    SQ1 = ([(1, kc) for kc in range(8)]
           + [(2, kc) for kc in range(4)]
           + [(0, kc) for kc in range(8, 16)]
           + [(3, kc) for kc in range(4)]
           + [(2, kc) for kc in range(4, 8)])
    SQ2 = ([(1, kc) for kc in range(8, 16)]
           + [(3, kc) for kc in range(4, 8)]
           + [(2, kc) for kc in range(8, 16)]
           + [(3, kc) for kc in range(8, 16)])
    EXPQ = [(0, kc) for kc in range(NPRE)] + SQ1 + SQ2
    exp_pos = {hk: i for i, hk in enumerate(EXPQ)}
    PVQ = [(h, kc) for h in range(NH_CORE) for kc in range(NKC)]
    sq1_i = 0
    pv_i = 0
    emitted_s = NPRE
    vst_hi = -1  # highest V block emitted so far

    def pump_pv(max_n):
        nonlocal pv_i
        n = 0
        while n < max_n and pv_i < len(PVQ):
            h, kc = PVQ[pv_i]
            # gate on V-block availability and on the exp pipeline having
            # run ~3 chunks past this one (Tile waits enforce correctness;
            # this just avoids head-of-line stalls)
            if kc > vst_hi or exp_pos[(h, kc)] > emitted_s - 4:
                return
            emit_pv(h, kc)
            pv_i += 1
            n += 1

    # F1: 8 rounds, each = 2 V blocks + 3 score chunks + ~2 PVs.
    # V blocks complete by round 7 so every head's PV pass (which needs
    # vst[15]) can finish early; exp backlog builds for the F2 drain.
    for r in range(8):
        for j in range(2):
            mg = 2 * r + j
            emit_vblock(mg, psv_pool)
            vst_hi = mg
            for _ in range(2 - j + (1 if r < 4 and j == 1 else 0)):
                if sq1_i < len(SQ1):
                    h, kc = SQ1[sq1_i]
                    sq1_i += 1
                    emit_schunk(h, kc, s_ps)
                    emitted_s += 1
            pump_pv(1)
        pump_pv(1)

    # F2: drain remaining scores + PVs (every vst/rope dep is satisfied)
    for j in range(len(SQ2)):
        emit_schunk(*SQ2[j], s_ps)
        emitted_s += 1
        pump_pv(2)
    emitted_s += 4  # release the exp-lag gate for the drain
    while pv_i < len(PVQ):
        emit_pv(*PVQ[pv_i])
        pv_i += 1
    flat_stack.close()
    s_stack.close()

    # ---- output projection, software-pipelined ------------------------
    # i0/i1 accumulations (aN0/aN1: heads 0-2) for the first four m-tiles
    # are emitted before any i2 so the PE keeps streaming while head 3's
    # DVE drain (aout/recB/aN writes) completes.
    po_stack = ExitStack()
    po_pool = po_stack.enter_context(
        tc.tile_pool(name="po_pool", bufs=8, space="PSUM"))
    pos = {}

    def o_head(m, i):
        if i == 0:
            pos[m] = (po_pool.tile([P, 512], F32, name=f"po{m}0", tag="po"),
                      po_pool.tile([P, 512], F32, name=f"po{m}1", tag="po"))
        po0, po1 = pos[m]
        lhs = wo_t[i][:, m * P:(m + 1) * P]
        nc.tensor.matmul(po0[:], lhs, aN[i][:, 0:512],
                         start=(i == 0), stop=(i == 2))
        nc.tensor.matmul(po1[:], lhs, aN[i][:, 512:1024],
                         start=(i == 0), stop=(i == 2))

    DMA_ENGS = [nc.sync, nc.gpsimd, nc.scalar]

    def o_finish(m):
        po0, po1 = pos.pop(m)
        ot = ot_pool.tile([P, TQ], PV_DT, name="ot", tag="ot")
        nc.vector.tensor_copy(ot[:, 0:512], po0[:])
        nc.scalar.copy(ot[:, 512:1024], po1[:])
        DMA_ENGS[m % 3].dma_start(out=oT[m * P:(m + 1) * P, :], in_=ot[:])

    for m in range(4):
        o_head(m, 0)
        o_head(m, 1)
    for m in range(MTILES):
        o_head(m, 2)
        o_finish(m)
        if m + 4 < MTILES:
            o_head(m + 4, 0)
            o_head(m + 4, 1)
    po_stack.close()


# ---------------------------------------------------------------- host side
def _rope_tables(coords, T):
    """Feature-major cos/sin tables [HD, T] with the sign fold.

    Row j < 48 of the rotated output is q[j]*cos_j - q[j+48]*sin_j and row
    j >= 48 is q[j]*cos_{j-48} + q[j-48]*sin_{j-48}; the device computes
    rot = q * cE + swap(q) * sE with swap(q)[j] = q[(j+48) % 96].
    """
    coords = np.asarray(coords, np.float32)
    inv_freq = (1.0 / (ROPE_BASE ** (np.arange(FREQ_PER_AXIS, dtype=np.float32)
                                     / FREQ_PER_AXIS))).astype(np.float32)
    ang = coords[:, :, None] * inv_freq[None, None, :]   # [T, 3, 16]
    ang = ang.reshape(T, ROPE_HALF)                      # [T, 48]
    sin = np.sin(ang).astype(np.float32).T               # [48, T]
    cos = np.cos(ang).astype(np.float32).T
    cE = np.concatenate([cos, cos], axis=0)              # [96, T]
    sE = np.concatenate([-sin, sin], axis=0)
    return (np.ascontiguousarray(cE).astype(QK_NP),
            np.ascontiguousarray(sE).astype(QK_NP))


def _make_in_maps(inputs):
    x = np.asarray(inputs["x"], np.float32)
    memory = np.asarray(inputs["memory"], np.float32)
    qc = np.asarray(inputs["query_coords"], np.float32)
    mc = np.asarray(inputs["memory_coords"], np.float32)
    Wq = np.asarray(inputs["Wq"], np.float32)
    Wk = np.asarray(inputs["Wk"], np.float32)
    Wv = np.asarray(inputs["Wv"], np.float32)
    Wo = np.asarray(inputs["Wo"], np.float32)
    bq = np.asarray(inputs["bq"], np.float32)
    bk = np.asarray(inputs["bk"], np.float32)

    WqT = np.ascontiguousarray(Wq.T).astype(QK_NP)   # [in, out]
    WkT = np.ascontiguousarray(Wk.T).astype(QK_NP)
    WvT = np.ascontiguousarray(Wv.T).astype(PV_NP)
    WoT = np.ascontiguousarray(Wo.T).astype(PV_NP)

    per_batch = []
    for b in range(B):
        cqE, sqE = _rope_tables(qc[b], TQ)
        ckE, skE = _rope_tables(mc[b], TK)
        entry = {
            "xT": np.ascontiguousarray(x[b].T).astype(QK_NP),
            "memT": np.ascontiguousarray(memory[b].T).astype(QK_NP),
            "cqE": cqE, "sqE": sqE, "ckE": ckE, "skE": skE,
        }
        per_batch.append(entry)

    in_maps = []
    for core in range(N_CORES):
        b, g = divmod(core, NH_CORE)
        sl = slice(g * HG, (g + 1) * HG)
        m = dict(per_batch[b])
        m["wqT"] = np.ascontiguousarray(WqT[:, sl])
        m["wkT"] = np.ascontiguousarray(WkT[:, sl])
        m["wvT"] = np.ascontiguousarray(WvT[:, sl])
        m["woT"] = np.ascontiguousarray(WoT[sl, :])
        m["bqP"] = np.ascontiguousarray(bq[sl].reshape(3, 128).T)
        m["bkP"] = np.ascontiguousarray(bk[sl].reshape(3, 128).T)
        in_maps.append(m)
    return in_maps


def _assemble(results, inputs):
    Wo = np.asarray(inputs["Wo"], np.float32)
    bv = np.asarray(inputs["bv"], np.float32)
    bo = np.asarray(inputs["bo"], np.float32)
    cvec = (bv @ Wo.T + bo).astype(np.float32)   # exact: attn rows sum to 1
    out = np.empty((B, TQ, D), np.float32)
    for b in range(B):
        acc = np.zeros((D, TQ), np.float32)
        for g in range(NH_CORE):
            acc += np.asarray(results[b * NH_CORE + g]["oT"], np.float32)
        out[b] = acc.T + cvec
    return out


_NC_CACHE = None


def _get_nc():
    global _NC_CACHE
    if _NC_CACHE is None:
        _NC_CACHE = _build_nc()
    return _NC_CACHE


_RUNNER = None


def _get_runner():
    """Reusable jitted PJRT executable (same lowering run_bass_kernel_spmd
    uses under axon) so repeated kernel() calls skip recompilation."""
    global _RUNNER
    if _RUNNER is not None:
        return _RUNNER
    import jax
    from jax.sharding import Mesh, PartitionSpec
    try:
        from jax.experimental.shard_map import shard_map
    except ImportError:
        from jax import shard_map
    from concourse import bass2jax

    nc = _get_nc()
    bass2jax.install_neuronx_cc_hook()
    partition_name = (nc.partition_id_tensor.name
                      if nc.partition_id_tensor else None)
    in_names, out_names, out_avals, zero_outs = [], [], [], []
    for alloc in nc.m.functions[0].allocations:
        if not isinstance(alloc, mybir.MemoryLocationSet):
            continue
        name = alloc.memorylocations[0].name
        if alloc.kind == "ExternalInput":
            if name != partition_name:
                in_names.append(name)
        elif alloc.kind == "ExternalOutput":
            out_names.append(name)
            shape = tuple(alloc.tensor_shape)
            dtype = mybir.dt.np(alloc.dtype)
            out_avals.append(jax.core.ShapedArray(shape, dtype))
            zero_outs.append(np.zeros(shape, dtype))
    n_params = len(in_names)
    all_in = list(in_names) + list(out_names)
    if partition_name is not None:
        all_in.append(partition_name)

    def _b(*args):
        operands = list(args)
        if partition_name is not None:
            operands.append(bass2jax.partition_id_tensor())
        return tuple(bass2jax._bass_exec_p.bind(
            *operands, out_avals=tuple(out_avals), in_names=tuple(all_in),
            out_names=tuple(out_names), lowering_input_output_aliases=(),
            sim_require_finite=True, sim_require_nnan=True, nc=nc))

    devices = jax.devices()[:N_CORES]
    mesh = Mesh(np.asarray(devices), ("core",))
    nio = n_params + len(out_avals)
    fn = jax.jit(shard_map(_b, mesh=mesh,
                           in_specs=(PartitionSpec("core"),) * nio,
                           out_specs=(PartitionSpec("core"),) * len(out_avals),
                           check_rep=False), keep_unused=True)

    def run(in_maps):
        per_core = [[np.asarray(m[n]) for n in in_names] for m in in_maps]
        concat_in = [np.concatenate([per_core[c][i] for c in range(N_CORES)],
                                    axis=0) for i in range(n_params)]
        concat_zeros = [np.zeros((N_CORES * z.shape[0], *z.shape[1:]), z.dtype)
                        for z in zero_outs]
        outs = fn(*concat_in, *concat_zeros)
        return [
            {name: np.asarray(outs[i]).reshape(N_CORES, *out_avals[i].shape)[c]
             for i, name in enumerate(out_names)}
            for c in range(N_CORES)
        ]

    _RUNNER = run
    return run


_CALLED = False


def kernel(**inputs) -> np.ndarray:
    """Full-input entry point: shards across 8 NeuronCores, runs the Bass
    kernel, gathers and unshards. First call uses run_bass_kernel_spmd
    (compile + run); later calls reuse the cached executable."""
    global _CALLED
    in_maps = _make_in_maps(inputs)
    if not _CALLED:
        _CALLED = True
        nc = _get_nc()
        res = run_bass_kernel_spmd(nc, in_maps, list(range(N_CORES)))
        results = res.results
    else:
        results = _get_runner()(in_maps)
    return _assemble(results, inputs)

